# revision 51
# baseline (speedup 1.0000x reference)
"""Fused multi-head attention block (QKV proj + RMSNorm + 2D RoPE + softmax
attention + out proj) for Trainium2, data-parallel over batch on 8 NeuronCores.

Layout strategy per core (one batch element, N=1024 tokens, D=1024, H=16, hd=64):
  - All matmul operands are bf16 (PSUM accumulation fp32).  fp32/fp32r moving
    operands stream at 2 cycles/element on the PE; bf16 streams at 1, and bf16
    stationary tiles get fast-weight-load.  Tolerance (2e-2) has ~40x headroom
    over the measured tf32 error, bf16 lands ~2-3e-3.
  - x is PE-transposed to xT [D, N] once (fp32 transpose, cast to bf16 on the
    PSUM->SBUF copy).
  - Q,K are produced transposed ("qkT" [feat, n]) so attention scores need no
    further transposes; V is produced in natural [n, feat] layout per head
    PAIR [v_even(64) | ones | v_odd(64)] so both heads' AV matmuls share one
    denominator ones-column.
  - RMSNorm 1/sigma_q is computed BROADCAST across all 128 partitions by a
    block-diagonal ones matmul (stationary [128,128]) -> no DMA broadcast of
    rstd_q is needed; 1/(8 sigma_k) is folded into the exp() scale operand.
  - RoPE rotate-half runs as two DVE table-multiplies plus a PE swap-matrix
    matmul (the +/- signs and q/k_scale are folded into host-built tables).
  - Softmax denominator falls out of the AV matmul ones-column; the two den
    rows round-trip through DRAM reshaped to [64,16] so one DVE reciprocal
    covers both heads on 64 lanes, then the reciprocals are DMA-broadcast
    across 64 partitions.  Both heads' AV tiles are
    [num(0..63) | den(64)] at base partition 0 (PE out base must be 0/32/64);
    the odd head's normalized tile is lane-shifted into oT rows 64..127 by a
    small SBUF->SBUF DMA (no DRAM round-trip for oT).
  - oT stays resident in SBUF and feeds the output projection directly.
Softmax skips max-subtraction: after RMSNorm ||q||<=8, ||k||<=8 so logits are
within [-64,64]*hd^-0.5 = [-8,8], safely inside fp32/bf16 exp range.
"""

import sys

sys.path.insert(0, "/opt/trn_rl_repo")

import numpy as np

_BUILT = None

B, N, D = 8, 1024, 1024
H, HD = 16, 64
P = 128
NB = 2          # free-dim blocks of 512 over n
FB = 512        # matmul free-dim block
VB = 256        # V-projection free-dim block
KT = D // P     # 8 contraction chunks
NT = N // P     # 8 n-chunks
VW = 2 * HD + 2  # vaug row stride per head pair: [v_even(64) | ones | v_odd(64) | ones]
THETA = 10000.0
EPS = 1e-6


def _rope_tables():
    side = int(np.sqrt(N))
    dq = HD // 4
    inv_freq = 1.0 / (THETA ** (np.arange(dq, dtype=np.float32) / dq))
    ang = np.arange(side, dtype=np.float32)[:, None] * inv_freq[None, :]
    row = np.broadcast_to(ang[:, None, :], (side, side, dq)).reshape(N, dq)
    col = np.broadcast_to(ang[None, :, :], (side, side, dq)).reshape(N, dq)
    angles = np.concatenate([row, col], axis=-1)  # [N, 32]
    return np.cos(angles), np.sin(angles)


def _perm_d(row):
    """Tile row -> dim-within-head, interleaved rotate-half pairing.

    Within each 64-row half (one head), rows (2j, 2j+1) hold dims (j, j+32),
    so the RoPE rotate-half partner is always the adjacent row and the swap
    is expressible as a DVE stream_shuffle (a within-32-quadrant permute)."""
    r = row % HD
    return (r % 2) * 32 + r // 2


def _build_tables():
    """cosF/sinF' [128, N] for a 2-head tile in interleaved pair layout.
    cosF[i] = cos(a_{(i%64)//2});  sinF'[i] = ±sin(a_{(i%64)//2}) with + on
    even rows (the d<32 member of the pair) and - on odd rows."""
    cos, sin = _rope_tables()  # [N, 32] each
    cosF = np.empty((P, N), np.float32)
    sinF = np.empty((P, N), np.float32)
    for i in range(P):
        a = (i % HD) // 2     # angle index
        cosF[i] = cos[:, a]
        sinF[i] = sin[:, a] * (1.0 if i % 2 == 0 else -1.0)
    return cosF, sinF


def _build_program(zero_bias=True):
    import concourse.bass as bass
    import concourse.mybir as mybir
    import concourse.tile as tile
    from concourse import bacc
    from concourse.bass import ds

    # Keep every ACT function this kernel uses (ln, exp, copy) in a single
    # table set so the table-load pass emits exactly one load instead of
    # thrashing between sets (~2.7us per switch).
    if not getattr(bacc, "_act_tables_patched", False):
        _orig_get_tables = bacc.get_activation_tables

        def _only_lnexp(arch):
            import concourse.mybir as _mb
            tabs = _orig_get_tables(arch)
            if "natural_log_exp_and_others" not in tabs:
                return tabs
            steer = set()
            for fname in ("Exp", "Ln", "Copy", "Identity", "Square"):
                steer.add(getattr(_mb.ActivationFunctionType, fname))
            out = {}
            for name, funcs in tabs.items():
                if name == "natural_log_exp_and_others":
                    out[name] = funcs
                else:
                    out[name] = funcs - steer
            return out

        bacc.get_activation_tables = _only_lnexp
        bacc._act_tables_patched = True

    BF16 = mybir.dt.bfloat16
    FP32 = mybir.dt.float32
    AF = mybir.ActivationFunctionType

    nc = bacc.Bacc("TRN2", target_bir_lowering=False, debug=False, num_devices=8)

    x = nc.dram_tensor("x", [N, D], BF16, kind="ExternalInput").ap()
    wqk_cols_d = nc.dram_tensor("wqk_cols", [2 * KT, P, KT, P], BF16, kind="ExternalInput").ap()
    wv_d = nc.dram_tensor("wv", [D, D], BF16, kind="ExternalInput").ap()
    wout_d = nc.dram_tensor("wout", [D, D], BF16, kind="ExternalInput").ap()
    bqkv_cols_d = nc.dram_tensor("bqkv_cols", [P, 2 * KT], FP32, kind="ExternalInput").ap()
    bqkv_v_d = nc.dram_tensor("bqkv_v", [1, D], BF16, kind="ExternalInput").ap()
    bout_d = nc.dram_tensor("bout_r", [1, D], BF16, kind="ExternalInput").ap()
    cosf_d = nc.dram_tensor("cosf", [P, N], BF16, kind="ExternalInput").ap()
    sinf_d = nc.dram_tensor("sinf", [P, N], BF16, kind="ExternalInput").ap()
    ident_d = nc.dram_tensor("ident", [P, P], FP32, kind="ExternalInput").ap()
    identb_d = nc.dram_tensor("identb", [P, P], BF16, kind="ExternalInput").ap()
    ones2qw_d = nc.dram_tensor("ones2qw", [P, P], BF16, kind="ExternalInput").ap()
    ones2kw_d = nc.dram_tensor("ones2kw", [P, P], BF16, kind="ExternalInput").ap()
    ones1_d = nc.dram_tensor("ones1", [1, P], BF16, kind="ExternalInput").ap()
    out = nc.dram_tensor("out", [N, D], FP32, kind="ExternalOutput").ap()
    rec_d = nc.dram_tensor("rec_scratch", [H, N], FP32).ap()

    with tile.TileContext(nc) as tc:
        with tc.tile_pool(name="big", bufs=1) as big, \
             tc.tile_pool(name="tab", bufs=1) as tab, \
             tc.tile_pool(name="wo", bufs=17) as wop:
            qkT = big.tile([P, 2 * KT, N], BF16)      # tile t: heads 2t,2t+1
            vaug = big.tile([P, NT, KT, VW], BF16)    # V per head pair + shared ones col
            oT = big.tile([P, KT, N], BF16)           # normalized attention out, [feat, n]

            cosf = tab.tile([P, N], BF16)
            sinf = tab.tile([P, N], BF16)
            ident = tab.tile([P, P], FP32)
            identb = tab.tile([P, P], BF16)
            ones2qw = tab.tile([P, P], BF16)
            ones2kw = tab.tile([P, P], BF16)
            ones1 = tab.tile([1, P], BF16)
            bqkv_cols = tab.tile([P, 2 * KT], FP32)
            bqkv_v = tab.tile([1, D], BF16)
            bout_t = tab.tile([1, D], BF16)
            eps_t = tab.tile([P, 1], FP32)
            eps64_t = tab.tile([P, 1], FP32)
            zero_t = tab.tile([P, 1], FP32)

            with tc.tile_pool(name="xTp", bufs=1) as xTp, \
                 tc.tile_pool(name="wv", bufs=17) as wvp, \
                 tc.tile_pool(name="wqk", bufs=4) as wqkp:
                xT = xTp.tile([P, KT, N], BF16)
                wv_pf = {}
                wcol_pf = {}

                # -------- Phase 1a: xT = transpose(x), bf16 --------
                with tc.tile_pool(name="xsb", bufs=1) as xsbp, \
                     tc.tile_pool(name="psxp", bufs=3, space="PSUM") as psxp:
                    # x arrives pre-cast to bf16 from the host: half the
                    # HBM traffic and no on-chip cast chain
                    xb_sb = xsbp.tile([P, NT, D], BF16)
                    nc.sync.dma_start(out=identb, in_=identb_d)
                    for no in range(NT):
                        eng = nc.scalar if no % 2 == 0 else nc.sync
                        eng.dma_start(out=xb_sb[:, no, :], in_=x[ds(no * P, P), :])
                    # prefetch the V weights for fb=0 and the first two qkT
                    # weight tiles ahead of the constant tables
                    for k in range(KT):
                        wvt = wvp.tile([P, FB], BF16, tag="wv")
                        nc.scalar.dma_start(
                            out=wvt, in_=wv_d[ds(k * P, P), 0:FB])
                        wv_pf[k] = wvt
                    for t in (0, KT):
                        wc = wqkp.tile([P, KT, P], BF16, tag="wc")
                        nc.scalar.dma_start(out=wc, in_=wqk_cols_d[t])
                        wcol_pf[t] = wc
                    for dst, src in [(cosf, cosf_d), (sinf, sinf_d),
                                     (ident, ident_d),
                                     (ones2qw, ones2qw_d),
                                     (ones2kw, ones2kw_d), (ones1, ones1_d),
                                     (bqkv_cols, bqkv_cols_d), (bqkv_v, bqkv_v_d),
                                     (bout_t, bout_d)]:
                        nc.sync.dma_start(out=dst, in_=src)
                    # ones columns of vaug via memset (a DMA from a [P,NT,KT,1]
                    # DRAM tensor costs ~7us of 2-byte descriptor generation)
                    nc.vector.memset(vaug[:, :, :, HD:HD + 1], 1.0)
                    nc.vector.memset(vaug[:, :, :, VW - 1:VW], 1.0)
                    nc.vector.memset(eps_t, EPS)
                    nc.vector.memset(eps64_t, EPS * HD)
                    nc.vector.memset(zero_t, 0.0)
                    for half in range(NB):
                        for k in range(KT):
                            pxt = psxp.tile([P, FB], BF16, tag="pxt")
                            for j in range(4):
                                nc.tensor.transpose(
                                    pxt[:, ds(j * P, P)],
                                    xb_sb[:, 4 * half + j, ds(k * P, P)],
                                    identb,
                                )
                            dst = xT[:, k, ds(half * FB, FB)]
                            if k % 2 == 0:
                                nc.vector.tensor_copy(out=dst, in_=pxt)
                            else:
                                nc.scalar.copy(out=dst, in_=pxt)

                # -------- V projection (512-wide blocks) --------
                with tc.tile_pool(name="psv", bufs=3, space="PSUM") as psv:
                    for fb in range(D // FB):
                        wvs = []
                        for k in range(KT):
                            if fb == 0:
                                wvs.append(wv_pf.pop(k))
                                continue
                            wv = wvp.tile([P, FB], BF16, tag="wv")
                            nc.sync.dma_start(
                                out=wv, in_=wv_d[ds(k * P, P), ds(fb * FB, FB)])
                            wvs.append(wv)
                        for mc in range(NT):
                            pv = psv.tile([P, FB], FP32, tag="pv")
                            for k in range(KT):
                                nc.tensor.matmul(
                                    pv, xT[:, k, ds(mc * P, P)], wvs[k],
                                    start=(k == 0),
                                    stop=(zero_bias and k == KT - 1))
                            if not zero_bias:
                                nc.tensor.matmul(
                                    pv, ones1, bqkv_v[:, ds(fb * FB, FB)],
                                    start=False, stop=True)
                            # scatter the 8 heads into pair slots: even member
                            # at cols 0:64, odd member at cols 65:129
                            pv2 = pv.rearrange("p (hp he d) -> p hp he d", he=2, d=HD)
                            if mc % 2 == 0:
                                nc.vector.tensor_copy(
                                    out=vaug[:, mc, ds(4 * fb, 4), 0:HD],
                                    in_=pv2[:, :, 0, :])
                                nc.vector.tensor_copy(
                                    out=vaug[:, mc, ds(4 * fb, 4), HD + 1:VW - 1],
                                    in_=pv2[:, :, 1, :])
                            else:
                                nc.scalar.copy(
                                    out=vaug[:, mc, ds(4 * fb, 4), 0:HD],
                                    in_=pv2[:, :, 0, :])
                                nc.scalar.copy(
                                    out=vaug[:, mc, ds(4 * fb, 4), HD + 1:VW - 1],
                                    in_=pv2[:, :, 1, :])

                # -------- fused per-head-pair pipeline --------
                with tc.tile_pool(name="sq", bufs=3) as sqp, \
                     tc.tile_pool(name="sg", bufs=3) as sgp, \
                     tc.tile_pool(name="rs", bufs=3) as rsp, \
                     tc.tile_pool(name="uc", bufs=7) as ucp, \
                     tc.tile_pool(name="ex", bufs=5) as exp_p, \
                     tc.tile_pool(name="avs", bufs=6) as avsp, \
                     tc.tile_pool(name="rcp", bufs=6) as rcp, \
                     tc.tile_pool(name="Rp", bufs=6) as rp, \
                     tc.tile_pool(name="ot1p", bufs=4) as otp, \
                     tc.tile_pool(name="pssp", bufs=2, space="PSUM") as pssp, \
                     tc.tile_pool(name="psmm", bufs=2, space="PSUM") as psmm, \
                     tc.tile_pool(name="psav", bufs=2, space="PSUM") as psav:

                    def emit_proj_stats(pg):
                        rs_tiles = {}
                        for t in (pg, KT + pg):
                            if t in wcol_pf:
                                wcol = wcol_pf.pop(t)
                            else:
                                wcol = wqkp.tile([P, KT, P], BF16, tag="wc")
                                nc.sync.dma_start(out=wcol, in_=wqk_cols_d[t])
                            pss = pssp.tile([P, 2 * FB], FP32, tag="sp")
                            for nb in range(NB):
                                sl = ds(nb * FB, FB)
                                pm = psmm.tile([P, FB], FP32, tag="mm")
                                for k in range(KT):
                                    nc.tensor.matmul(
                                        pm, wcol[:, k, :], xT[:, k, sl],
                                        start=(k == 0), stop=(k == KT - 1))
                                nc.vector.tensor_scalar_add(
                                    out=qkT[:, t, sl], in0=pm,
                                    scalar1=bqkv_cols[:, t:t + 1])
                                sq = sqp.tile([P, FB], BF16, tag="sq")
                                nc.vector.tensor_mul(
                                    out=sq, in0=qkT[:, t, sl], in1=qkT[:, t, sl])
                                # sumsq/mean, broadcast across all 128 rows by the
                                # block-diagonal stationary operand
                                nc.tensor.matmul(
                                    pss[:, sl], ones2qw if t < KT else ones2kw, sq,
                                    start=True, stop=True)
                            # q: 1/sigma_q      (mean-based, eps)
                            # k: 1/(8 sigma_k)  (sumsq-based, 64*eps)
                            sg = sgp.tile([P, N], FP32, tag="sg")
                            nc.scalar.activation(
                                out=sg, in_=pss, func=AF.Ln, scale=1.0,
                                bias=eps_t if t < KT else eps64_t)
                            rs_tiles[t] = sg
                        return rs_tiles

                    def emit_stats_exp(sg_tiles):
                        # deferred so the att exps of the previous head pair
                        # aren't queued behind these on the scalar engine
                        rs_tiles = {}
                        for t, sg in sg_tiles.items():
                            rs = rsp.tile([P, N], BF16, tag="rs")
                            nc.scalar.activation(
                                out=rs, in_=sg, func=AF.Exp, scale=-0.5, bias=zero_t)
                            rs_tiles[t] = rs
                        return rs_tiles

                    SWAP_MASK = [i ^ 1 for i in range(32)]

                    def emit_rope(pg, rs_tiles):
                        # rotate-half entirely on the DVE: the interleaved
                        # pair layout makes the partner the adjacent row, a
                        # stream_shuffle mask.  rstd (per-column) commutes
                        # with the rotation so it is applied first.
                        for t in (pg, KT + pg):
                            qn = ucp.tile([P, N], BF16, tag="uc")
                            nc.vector.tensor_mul(
                                out=qn, in0=qkT[:, t, :], in1=rs_tiles[t])
                            u = ucp.tile([P, N], BF16, tag="uc")
                            nc.vector.tensor_mul(out=u, in0=qn, in1=sinf)
                            us = ucp.tile([P, N], BF16, tag="uc")
                            nc.vector.stream_shuffle(out=us, in_=u, mask=SWAP_MASK)
                            c = ucp.tile([P, N], BF16, tag="uc")
                            nc.vector.tensor_mul(out=c, in0=qn, in1=cosf)
                            nc.vector.tensor_add(out=qkT[:, t, :], in0=us, in1=c)

                    def emit_att_mm(pg):
                        # attention matmuls for heads (2pg, 2pg+1); rstd_k is
                        # already folded into kT so exp() needs no scale
                        # operand and both heads share one merged exp.
                        # Returns the staged AV tiles for emit_att_norm.
                        staged = []
                        for nb in range(NB):
                            sl = ds(nb * FB, FB)
                            av0 = psav.tile([P, FB], FP32, tag="av")
                            av1 = psav.tile([P, FB], FP32, tag="av")
                            es = {}

                            def emit_avs(m):
                                e = es.pop(m)
                                nc.tensor.matmul(
                                    av0[0:HD + 1, :], vaug[:, m, pg, 0:HD + 1],
                                    e[:, 0:FB],
                                    start=(m == 0), stop=(m == NT - 1))
                                nc.tensor.matmul(
                                    av1[0:HD + 1, :], vaug[:, m, pg, HD + 1:VW],
                                    e[:, FB:2 * FB],
                                    start=(m == 0), stop=(m == NT - 1))

                            for mc in range(NT):
                                sp = pssp.tile([P, 2 * FB], FP32, tag="sp")
                                nc.tensor.matmul(
                                    sp[:, 0:FB], qkT[0:HD, KT + pg, ds(mc * P, P)],
                                    qkT[0:HD, pg, sl], start=True, stop=True)
                                nc.tensor.matmul(
                                    sp[:, FB:2 * FB], qkT[HD:P, KT + pg, ds(mc * P, P)],
                                    qkT[HD:P, pg, sl], start=True, stop=True)
                                e = exp_p.tile([P, 2 * FB], BF16, tag="e")
                                nc.scalar.activation(
                                    out=e, in_=sp, func=AF.Exp, scale=1.0, bias=zero_t)
                                es[mc] = e
                                if mc >= 1:
                                    emit_avs(mc - 1)
                            emit_avs(NT - 1)

                            # stage AV out of PSUM so the banks free immediately
                            avs0 = avsp.tile([HD + 1, FB], FP32, tag="avs")
                            avs1 = avsp.tile([HD + 1, FB], FP32, tag="avs")
                            nc.vector.tensor_copy(out=avs0, in_=av0[0:HD + 1, :])
                            nc.vector.tensor_copy(out=avs1, in_=av1[0:HD + 1, :])
                            staged.append((avs0, avs1))
                        return staged

                    def emit_att_norm(pg, staged):
                        # normalize the staged AV tiles; emitted AFTER rope(pg)
                        # so the DMA-latency-bound chain here doesn't block
                        # rope's DVE ops in the in-order queues
                        for nb in range(NB):
                            sl = ds(nb * FB, FB)
                            avs0, avs1 = staged[nb]
                            # spread the two [1,512] den rows across 64 DVE
                            # lanes (SBUF->SBUF reshape DMA) so one reciprocal
                            # covers both heads at 16 elems/lane
                            dg = rcp.tile([HD, 16], FP32, tag="dg")
                            nc.gpsimd.dma_start(out=dg[0:32, :], in_=avs0[HD:HD + 1, :])
                            nc.gpsimd.dma_start(out=dg[32:HD, :], in_=avs1[HD:HD + 1, :])
                            rg = rcp.tile([HD, 16], FP32, tag="rg")
                            nc.vector.reciprocal(out=rg, in_=dg)
                            nc.gpsimd.dma_start(
                                out=rec_d[2 * pg:2 * pg + 1, sl], in_=rg[0:32, :])
                            nc.gpsimd.dma_start(
                                out=rec_d[2 * pg + 1:2 * pg + 2, sl], in_=rg[32:HD, :])
                            R0 = rp.tile([HD, FB], FP32, tag="R")
                            R1 = rp.tile([HD, FB], FP32, tag="R")
                            nc.gpsimd.dma_start(
                                out=R0,
                                in_=rec_d[2 * pg:2 * pg + 1, sl].broadcast_to([HD, FB]))
                            nc.gpsimd.dma_start(
                                out=R1,
                                in_=rec_d[2 * pg + 1:2 * pg + 2, sl].broadcast_to([HD, FB]))
                            nc.vector.tensor_mul(
                                out=oT[0:HD, pg, sl], in0=avs0[0:HD, :], in1=R0)
                            ot1 = otp.tile([HD, FB], BF16, tag="ot1")
                            nc.vector.tensor_mul(
                                out=ot1, in0=avs1[0:HD, :], in1=R1)
                            nc.gpsimd.dma_start(out=oT[HD:P, pg, sl], in_=ot1)

                    wos_all = []

                    for pg in range(KT):
                        sg_tiles = emit_proj_stats(pg)
                        # att(pg-1) matmuls before rope(pg): they are ready
                        # (inputs roped last iteration) while rope(pg) waits
                        # on the stats->Ln->Exp->DVE chain; the normalize
                        # tail goes after rope so its DMA round-trips don't
                        # block rope's DVE ops
                        staged = emit_att_mm(pg - 1) if pg >= 1 else None
                        rs_tiles = emit_stats_exp(sg_tiles)
                        emit_rope(pg, rs_tiles)
                        if staged is not None:
                            emit_att_norm(pg - 1, staged)
                    # prefetch output-projection weights under the last att
                    for ob in range(NB):
                        for k in range(KT):
                            wo = wop.tile([P, FB], BF16, tag="wo")
                            nc.sync.dma_start(
                                out=wo, in_=wout_d[ds(k * P, P), ds(ob * FB, FB)])
                            wos_all.append(wo)
                    staged = emit_att_mm(KT - 1)
                    emit_att_norm(KT - 1, staged)

            # -------- Phase 4: output projection (oT resident in SBUF) ----
            # ob interleaved inside nch so the 4MB of output DMA spreads
            # evenly instead of bunching at the end
            with tc.tile_pool(name="oout", bufs=3) as ooutp, \
                 tc.tile_pool(name="pso", bufs=3, space="PSUM") as pso:
                for nch in range(NT):
                    for ob in range(NB):
                        po = pso.tile([P, FB], FP32, tag="po")
                        for k in range(KT):
                            nc.tensor.matmul(
                                po, oT[:, k, ds(nch * P, P)], wos_all[ob * KT + k],
                                start=(k == 0),
                                stop=(zero_bias and k == KT - 1))
                        if not zero_bias:
                            nc.tensor.matmul(po, ones1, bout_t[:, ds(ob * FB, FB)],
                                             start=False, stop=True)
                        osb = ooutp.tile([P, FB], FP32, tag="osb")
                        if ob % 2 == 0:
                            nc.scalar.copy(out=osb, in_=po)
                        else:
                            nc.vector.tensor_copy(out=osb, in_=po)
                        nc.scalar.dma_start(
                            out=out[ds(nch * P, P), ds(ob * FB, FB)], in_=osb)

    nc.compile()
    return nc


def _host_inputs(Wqkv, bqkv, Wout, bout, q_scale, k_scale):
    import ml_dtypes
    BF = ml_dtypes.bfloat16

    cosF, sinF = _build_tables()

    ident = np.eye(P, dtype=np.float32)
    ones1 = np.ones((1, P), np.float32)

    # Fold q/k_scale into the Q/K projection columns; the RMSNorm variance of
    # the *unscaled* q is then recovered with a 1/scale^2-weighted reduction.
    qs = q_scale.astype(np.float32)
    ks = k_scale.astype(np.float32)
    W = Wqkv.astype(np.float32).copy()
    b = bqkv.astype(np.float32).copy()
    qcol = np.tile(qs, H)      # [D] scale per q feature
    kcol = np.tile(ks, H)
    W[:, 0:D] *= qcol[None, :]
    W[:, D:2 * D] *= kcol[None, :]
    b[0:D] *= qcol
    b[D:2 * D] *= kcol

    # wqk_cols[t, ki, ko, f] = W[ko*128+ki, t*128+perm(f)] -- q/k feature
    # rows in interleaved rotate-half pair order
    perm = np.array([(f // HD) * HD + _perm_d(f) for f in range(P)])
    wqk = np.ascontiguousarray(
        W[:, :2 * D].reshape(KT, P, 2 * KT, P).transpose(2, 1, 0, 3))[:, :, :, perm]
    wqk = np.ascontiguousarray(wqk)

    # block-diagonal stationary for broadcast q sumsq: col j weighted by
    # 1/(HD*scale^2) over the rows of j's head half
    iq = 1.0 / (qs * qs)
    iq_rows = np.array([iq[_perm_d(r)] for r in range(P)], np.float32)
    ones2qw = np.zeros((P, P), np.float32)
    for j in range(P):
        if j < HD:
            ones2qw[0:HD, j] = iq_rows[0:HD] / HD
        else:
            ones2qw[HD:P, j] = iq_rows[HD:P] / HD

    # same but sumsq-weighted (no /HD): 1/sqrt(sumsq) = 1/(8 sigma_k) folds
    # the hd^-0.5 softmax scale into kT
    ik = 1.0 / (ks * ks)
    ik_rows = np.array([ik[_perm_d(r)] for r in range(P)], np.float32)
    ones2kw = np.zeros((P, P), np.float32)
    for j in range(P):
        if j < HD:
            ones2kw[0:HD, j] = ik_rows[0:HD]
        else:
            ones2kw[HD:P, j] = ik_rows[HD:P]

    bqkv_cols = np.ascontiguousarray(
        b[:2 * D].reshape(2 * KT, P).T[perm, :]).astype(np.float32)

    return {
        "wqk_cols": wqk.astype(BF),
        "wv": np.ascontiguousarray(W[:, 2 * D:]).astype(BF),
        "wout": Wout.astype(np.float32).astype(BF),
        "bqkv_cols": bqkv_cols,
        "bqkv_v": b[2 * D:].reshape(1, D).astype(BF),
        "bout_r": bout.reshape(1, D).astype(np.float32).astype(BF),
        "cosf": cosF.astype(BF), "sinf": sinF.astype(BF),
        "ident": ident, "identb": ident.astype(BF),
        "ones2qw": ones2qw.astype(BF), "ones2kw": ones2kw.astype(BF),
        "ones1": ones1.astype(BF),
    }


def _get_built(zero_bias):
    global _BUILT
    if not isinstance(_BUILT, dict):
        _BUILT = {}
    if zero_bias not in _BUILT:
        _BUILT[zero_bias] = _build_program(zero_bias=zero_bias)
    return _BUILT[zero_bias]


def kernel(x, Wqkv, bqkv, Wout, bout, q_scale, k_scale, _trace=False):
    from concourse.bass_utils import run_bass_kernel_spmd

    x = np.asarray(x, dtype=np.float32)
    shared = _host_inputs(np.asarray(Wqkv, np.float32), np.asarray(bqkv, np.float32),
                          np.asarray(Wout, np.float32), np.asarray(bout, np.float32),
                          np.asarray(q_scale, np.float32), np.asarray(k_scale, np.float32))
    import ml_dtypes
    xb = np.ascontiguousarray(x).astype(ml_dtypes.bfloat16)
    in_maps = [dict(shared, x=xb[c]) for c in range(B)]
    zb = bool(not np.any(np.asarray(bqkv)) and not np.any(np.asarray(bout)))
    nc = _get_built(zb)
    res = run_bass_kernel_spmd(nc, in_maps, core_ids=list(range(B)), trace=_trace)
    out = np.stack([res.results[c]["out"] for c in range(B)], axis=0)
    kernel.last_exec_time_ns = res.exec_time_ns
    kernel.last_results = res
    return out


# revision 52
# speedup vs baseline: 1.1933x; 1.1933x over previous
"""Fused multi-head attention block (QKV proj + RMSNorm + 2D RoPE + softmax
attention + out proj) for Trainium2, data-parallel over batch on 8 NeuronCores.

Layout strategy per core (one batch element, N=1024 tokens, D=1024, H=16, hd=64):
  - All matmul operands are bf16 (PSUM accumulation fp32).  fp32/fp32r moving
    operands stream at 2 cycles/element on the PE; bf16 streams at 1, and bf16
    stationary tiles get fast-weight-load.  Tolerance (2e-2) has ~40x headroom
    over the measured tf32 error, bf16 lands ~2-3e-3.
  - x is PE-transposed to xT [D, N] once (fp32 transpose, cast to bf16 on the
    PSUM->SBUF copy).
  - Q,K are produced transposed ("qkT" [feat, n]) so attention scores need no
    further transposes; V is produced in natural [n, feat] layout per head
    PAIR [v_even(64) | ones | v_odd(64)] so both heads' AV matmuls share one
    denominator ones-column.
  - RMSNorm 1/sigma_q is computed BROADCAST across all 128 partitions by a
    block-diagonal ones matmul (stationary [128,128]) -> no DMA broadcast of
    rstd_q is needed; 1/(8 sigma_k) is folded into the exp() scale operand.
  - RoPE rotate-half runs as two DVE table-multiplies plus a PE swap-matrix
    matmul (the +/- signs and q/k_scale are folded into host-built tables).
  - Softmax denominator falls out of the AV matmul ones-column; the two den
    rows round-trip through DRAM reshaped to [64,16] so one DVE reciprocal
    covers both heads on 64 lanes, then the reciprocals are DMA-broadcast
    across 64 partitions.  Both heads' AV tiles are
    [num(0..63) | den(64)] at base partition 0 (PE out base must be 0/32/64);
    the odd head's normalized tile is lane-shifted into oT rows 64..127 by a
    small SBUF->SBUF DMA (no DRAM round-trip for oT).
  - oT stays resident in SBUF and feeds the output projection directly.
Softmax skips max-subtraction: after RMSNorm ||q||<=8, ||k||<=8 so logits are
within [-64,64]*hd^-0.5 = [-8,8], safely inside fp32/bf16 exp range.
"""

import sys

sys.path.insert(0, "/opt/trn_rl_repo")

import numpy as np

_BUILT = None

B, N, D = 8, 1024, 1024
H, HD = 16, 64
P = 128
NB = 2          # free-dim blocks of 512 over n
FB = 512        # matmul free-dim block
VB = 256        # V-projection free-dim block
KT = D // P     # 8 contraction chunks
NT = N // P     # 8 n-chunks
VW = 2 * HD + 2  # vaug row stride per head pair: [v_even(64) | ones | v_odd(64) | ones]
THETA = 10000.0
EPS = 1e-6


def _rope_tables():
    side = int(np.sqrt(N))
    dq = HD // 4
    inv_freq = 1.0 / (THETA ** (np.arange(dq, dtype=np.float32) / dq))
    ang = np.arange(side, dtype=np.float32)[:, None] * inv_freq[None, :]
    row = np.broadcast_to(ang[:, None, :], (side, side, dq)).reshape(N, dq)
    col = np.broadcast_to(ang[None, :, :], (side, side, dq)).reshape(N, dq)
    angles = np.concatenate([row, col], axis=-1)  # [N, 32]
    return np.cos(angles), np.sin(angles)


def _perm_d(row):
    """Tile row -> dim-within-head, interleaved rotate-half pairing.

    Within each 64-row half (one head), rows (2j, 2j+1) hold dims (j, j+32),
    so the RoPE rotate-half partner is always the adjacent row and the swap
    is expressible as a DVE stream_shuffle (a within-32-quadrant permute)."""
    r = row % HD
    return (r % 2) * 32 + r // 2


def _build_tables():
    """cosF/sinF' [128, N] for a 2-head tile in interleaved pair layout.
    cosF[i] = cos(a_{(i%64)//2});  sinF'[i] = ±sin(a_{(i%64)//2}) with + on
    even rows (the d<32 member of the pair) and - on odd rows."""
    cos, sin = _rope_tables()  # [N, 32] each
    cosF = np.empty((P, N), np.float32)
    sinF = np.empty((P, N), np.float32)
    for i in range(P):
        a = (i % HD) // 2     # angle index
        cosF[i] = cos[:, a]
        sinF[i] = sin[:, a] * (1.0 if i % 2 == 0 else -1.0)
    return cosF, sinF


def _build_program(zero_bias=True):
    import concourse.bass as bass
    import concourse.mybir as mybir
    import concourse.tile as tile
    from concourse import bacc
    from concourse.bass import ds

    # Keep every ACT function this kernel uses (ln, exp, copy) in a single
    # table set so the table-load pass emits exactly one load instead of
    # thrashing between sets (~2.7us per switch).
    if not getattr(bacc, "_act_tables_patched", False):
        _orig_get_tables = bacc.get_activation_tables

        def _only_lnexp(arch):
            import concourse.mybir as _mb
            tabs = _orig_get_tables(arch)
            if "natural_log_exp_and_others" not in tabs:
                return tabs
            steer = set()
            for fname in ("Exp", "Ln", "Copy", "Identity", "Square"):
                steer.add(getattr(_mb.ActivationFunctionType, fname))
            out = {}
            for name, funcs in tabs.items():
                if name == "natural_log_exp_and_others":
                    out[name] = funcs
                else:
                    out[name] = funcs - steer
            return out

        bacc.get_activation_tables = _only_lnexp
        bacc._act_tables_patched = True

    BF16 = mybir.dt.bfloat16
    FP32 = mybir.dt.float32
    AF = mybir.ActivationFunctionType

    nc = bacc.Bacc("TRN2", target_bir_lowering=False, debug=False, num_devices=8)

    x = nc.dram_tensor("x", [N, D], BF16, kind="ExternalInput").ap()
    wqk_cols_d = nc.dram_tensor("wqk_cols", [2 * KT, P, KT, P], BF16, kind="ExternalInput").ap()
    wv_d = nc.dram_tensor("wv", [D, D], BF16, kind="ExternalInput").ap()
    wout_d = nc.dram_tensor("wout", [D, D], BF16, kind="ExternalInput").ap()
    bqkv_cols_d = nc.dram_tensor("bqkv_cols", [P, 2 * KT], FP32, kind="ExternalInput").ap()
    bqkv_v_d = nc.dram_tensor("bqkv_v", [1, D], BF16, kind="ExternalInput").ap()
    bout_d = nc.dram_tensor("bout_r", [1, D], BF16, kind="ExternalInput").ap()
    cosf_d = nc.dram_tensor("cosf", [P, N], BF16, kind="ExternalInput").ap()
    sinf_d = nc.dram_tensor("sinf", [P, N], BF16, kind="ExternalInput").ap()
    ident_d = nc.dram_tensor("ident", [P, P], FP32, kind="ExternalInput").ap()
    identb_d = nc.dram_tensor("identb", [P, P], BF16, kind="ExternalInput").ap()
    ones2qw_d = nc.dram_tensor("ones2qw", [P, P], BF16, kind="ExternalInput").ap()
    ones2kw_d = nc.dram_tensor("ones2kw", [P, P], BF16, kind="ExternalInput").ap()
    ones1_d = nc.dram_tensor("ones1", [1, P], BF16, kind="ExternalInput").ap()
    out = nc.dram_tensor("out", [N, D], FP32, kind="ExternalOutput").ap()
    rec_d = nc.dram_tensor("rec_scratch", [H, N], FP32).ap()

    with tile.TileContext(nc) as tc:
        with tc.tile_pool(name="big", bufs=1) as big, \
             tc.tile_pool(name="tab", bufs=1) as tab, \
             tc.tile_pool(name="wo", bufs=17) as wop:
            qkT = big.tile([P, 2 * KT, N], BF16)      # tile t: heads 2t,2t+1
            vaug = big.tile([P, NT, KT, VW], BF16)    # V per head pair + shared ones col
            oT = big.tile([P, KT, N], BF16)           # normalized attention out, [feat, n]

            cosf = tab.tile([P, N], BF16)
            sinf = tab.tile([P, N], BF16)
            ident = tab.tile([P, P], FP32)
            identb = tab.tile([P, P], BF16)
            ones2qw = tab.tile([P, P], BF16)
            ones2kw = tab.tile([P, P], BF16)
            ones1 = tab.tile([1, P], BF16)
            bqkv_cols = tab.tile([P, 2 * KT], FP32)
            bqkv_v = tab.tile([1, D], BF16)
            bout_t = tab.tile([1, D], BF16)
            eps_t = tab.tile([P, 1], FP32)
            eps64_t = tab.tile([P, 1], FP32)
            zero_t = tab.tile([P, 1], FP32)

            with tc.tile_pool(name="xTp", bufs=1) as xTp, \
                 tc.tile_pool(name="wv", bufs=17) as wvp, \
                 tc.tile_pool(name="wqk", bufs=4) as wqkp:
                xT = xTp.tile([P, KT, N], BF16)
                wv_pf = {}
                wcol_pf = {}

                # -------- Phase 1a: xT = transpose(x), bf16 --------
                with tc.tile_pool(name="xsb", bufs=1) as xsbp, \
                     tc.tile_pool(name="psxp", bufs=3, space="PSUM") as psxp:
                    # x arrives pre-cast to bf16 from the host: half the
                    # HBM traffic and no on-chip cast chain
                    xb_sb = xsbp.tile([P, NT, D], BF16)
                    nc.sync.dma_start(out=identb, in_=identb_d)
                    for no in range(NT):
                        eng = nc.scalar if no % 2 == 0 else nc.sync
                        eng.dma_start(out=xb_sb[:, no, :], in_=x[ds(no * P, P), :])
                    # prefetch the V weights for fb=0 and the first two qkT
                    # weight tiles ahead of the constant tables
                    for k in range(KT):
                        wvt = wvp.tile([P, FB], BF16, tag="wv")
                        nc.scalar.dma_start(
                            out=wvt, in_=wv_d[ds(k * P, P), 0:FB])
                        wv_pf[k] = wvt
                    for t in (0, KT):
                        wc = wqkp.tile([P, KT, P], BF16, tag="wc")
                        nc.scalar.dma_start(out=wc, in_=wqk_cols_d[t])
                        wcol_pf[t] = wc
                    for dst, src in [(cosf, cosf_d), (sinf, sinf_d),
                                     (ident, ident_d),
                                     (ones2qw, ones2qw_d),
                                     (ones2kw, ones2kw_d), (ones1, ones1_d),
                                     (bqkv_cols, bqkv_cols_d), (bqkv_v, bqkv_v_d),
                                     (bout_t, bout_d)]:
                        nc.sync.dma_start(out=dst, in_=src)
                    # ones columns of vaug via memset (a DMA from a [P,NT,KT,1]
                    # DRAM tensor costs ~7us of 2-byte descriptor generation)
                    nc.vector.memset(vaug[:, :, :, HD:HD + 1], 1.0)
                    nc.vector.memset(vaug[:, :, :, VW - 1:VW], 1.0)
                    nc.vector.memset(eps_t, EPS)
                    nc.vector.memset(eps64_t, EPS * HD)
                    nc.vector.memset(zero_t, 0.0)
                    for half in range(NB):
                        for k in range(KT):
                            pxt = psxp.tile([P, FB], BF16, tag="pxt")
                            for j in range(4):
                                nc.tensor.transpose(
                                    pxt[:, ds(j * P, P)],
                                    xb_sb[:, 4 * half + j, ds(k * P, P)],
                                    identb,
                                )
                            dst = xT[:, k, ds(half * FB, FB)]
                            if k % 2 == 0:
                                nc.vector.tensor_copy(out=dst, in_=pxt)
                            else:
                                nc.scalar.copy(out=dst, in_=pxt)

                # -------- V projection (512-wide blocks) --------
                with tc.tile_pool(name="psv", bufs=3, space="PSUM") as psv:
                    for fb in range(D // FB):
                        wvs = []
                        for k in range(KT):
                            if fb == 0:
                                wvs.append(wv_pf.pop(k))
                                continue
                            wv = wvp.tile([P, FB], BF16, tag="wv")
                            nc.sync.dma_start(
                                out=wv, in_=wv_d[ds(k * P, P), ds(fb * FB, FB)])
                            wvs.append(wv)
                        for mc in range(NT):
                            pv = psv.tile([P, FB], FP32, tag="pv")
                            for k in range(KT):
                                nc.tensor.matmul(
                                    pv, xT[:, k, ds(mc * P, P)], wvs[k],
                                    start=(k == 0),
                                    stop=(zero_bias and k == KT - 1))
                            if not zero_bias:
                                nc.tensor.matmul(
                                    pv, ones1, bqkv_v[:, ds(fb * FB, FB)],
                                    start=False, stop=True)
                            # scatter the 8 heads into pair slots: even member
                            # at cols 0:64, odd member at cols 65:129
                            pv2 = pv.rearrange("p (hp he d) -> p hp he d", he=2, d=HD)
                            if mc % 2 == 0:
                                nc.vector.tensor_copy(
                                    out=vaug[:, mc, ds(4 * fb, 4), 0:HD],
                                    in_=pv2[:, :, 0, :])
                                nc.vector.tensor_copy(
                                    out=vaug[:, mc, ds(4 * fb, 4), HD + 1:VW - 1],
                                    in_=pv2[:, :, 1, :])
                            else:
                                nc.scalar.copy(
                                    out=vaug[:, mc, ds(4 * fb, 4), 0:HD],
                                    in_=pv2[:, :, 0, :])
                                nc.scalar.copy(
                                    out=vaug[:, mc, ds(4 * fb, 4), HD + 1:VW - 1],
                                    in_=pv2[:, :, 1, :])

                # -------- fused per-head-pair pipeline --------
                with tc.tile_pool(name="sq", bufs=3) as sqp, \
                     tc.tile_pool(name="sg", bufs=3) as sgp, \
                     tc.tile_pool(name="rs", bufs=3) as rsp, \
                     tc.tile_pool(name="uc", bufs=7) as ucp, \
                     tc.tile_pool(name="ex", bufs=5) as exp_p, \
                     tc.tile_pool(name="avs", bufs=4) as avsp, \
                     tc.tile_pool(name="rcp", bufs=4) as rcp, \
                     tc.tile_pool(name="Rp", bufs=4) as rp, \
                     tc.tile_pool(name="ot1p", bufs=3) as otp, \
                     tc.tile_pool(name="pssp", bufs=2, space="PSUM") as pssp, \
                     tc.tile_pool(name="psmm", bufs=2, space="PSUM") as psmm, \
                     tc.tile_pool(name="psav", bufs=2, space="PSUM") as psav:

                    def emit_proj_stats(pg):
                        rs_tiles = {}
                        for t in (pg, KT + pg):
                            if t in wcol_pf:
                                wcol = wcol_pf.pop(t)
                            else:
                                wcol = wqkp.tile([P, KT, P], BF16, tag="wc")
                                nc.sync.dma_start(out=wcol, in_=wqk_cols_d[t])
                            pss = pssp.tile([P, 2 * FB], FP32, tag="sp")
                            for nb in range(NB):
                                sl = ds(nb * FB, FB)
                                pm = psmm.tile([P, FB], FP32, tag="mm")
                                for k in range(KT):
                                    nc.tensor.matmul(
                                        pm, wcol[:, k, :], xT[:, k, sl],
                                        start=(k == 0), stop=(k == KT - 1))
                                nc.vector.tensor_scalar_add(
                                    out=qkT[:, t, sl], in0=pm,
                                    scalar1=bqkv_cols[:, t:t + 1])
                                sq = sqp.tile([P, FB], BF16, tag="sq")
                                nc.vector.tensor_mul(
                                    out=sq, in0=qkT[:, t, sl], in1=qkT[:, t, sl])
                                # sumsq/mean, broadcast across all 128 rows by the
                                # block-diagonal stationary operand
                                nc.tensor.matmul(
                                    pss[:, sl], ones2qw if t < KT else ones2kw, sq,
                                    start=True, stop=True)
                            # q: 1/sigma_q      (mean-based, eps)
                            # k: 1/(8 sigma_k)  (sumsq-based, 64*eps)
                            sg = sgp.tile([P, N], FP32, tag="sg")
                            nc.scalar.activation(
                                out=sg, in_=pss, func=AF.Ln, scale=1.0,
                                bias=eps_t if t < KT else eps64_t)
                            rs_tiles[t] = sg
                        return rs_tiles

                    def emit_stats_exp(sg_tiles):
                        # deferred so the att exps of the previous head pair
                        # aren't queued behind these on the scalar engine
                        rs_tiles = {}
                        for t, sg in sg_tiles.items():
                            rs = rsp.tile([P, N], BF16, tag="rs")
                            nc.scalar.activation(
                                out=rs, in_=sg, func=AF.Exp, scale=-0.5, bias=zero_t)
                            rs_tiles[t] = rs
                        return rs_tiles

                    SWAP_MASK = [i ^ 1 for i in range(32)]

                    def emit_rope(pg, rs_tiles):
                        # rotate-half entirely on the DVE: the interleaved
                        # pair layout makes the partner the adjacent row, a
                        # stream_shuffle mask.  rstd (per-column) commutes
                        # with the rotation so it is applied first.
                        for t in (pg, KT + pg):
                            qn = ucp.tile([P, N], BF16, tag="uc")
                            nc.vector.tensor_mul(
                                out=qn, in0=qkT[:, t, :], in1=rs_tiles[t])
                            u = ucp.tile([P, N], BF16, tag="uc")
                            nc.vector.tensor_mul(out=u, in0=qn, in1=sinf)
                            us = ucp.tile([P, N], BF16, tag="uc")
                            nc.vector.stream_shuffle(out=us, in_=u, mask=SWAP_MASK)
                            c = ucp.tile([P, N], BF16, tag="uc")
                            nc.vector.tensor_mul(out=c, in0=qn, in1=cosf)
                            nc.vector.tensor_add(out=qkT[:, t, :], in0=us, in1=c)

                    def emit_att_mm(pg):
                        # attention matmuls for heads (2pg, 2pg+1); rstd_k is
                        # already folded into kT so exp() needs no scale
                        # operand and both heads share one merged exp.
                        # Returns the staged AV tiles for emit_att_norm.
                        staged = []
                        for nb in range(NB):
                            sl = ds(nb * FB, FB)
                            av0 = psav.tile([P, FB], FP32, tag="av")
                            av1 = psav.tile([P, FB], FP32, tag="av")
                            es = {}

                            def emit_avs(m):
                                e = es.pop(m)
                                nc.tensor.matmul(
                                    av0[0:HD + 1, :], vaug[:, m, pg, 0:HD + 1],
                                    e[:, 0:FB],
                                    start=(m == 0), stop=(m == NT - 1))
                                nc.tensor.matmul(
                                    av1[0:HD + 1, :], vaug[:, m, pg, HD + 1:VW],
                                    e[:, FB:2 * FB],
                                    start=(m == 0), stop=(m == NT - 1))

                            for mc in range(NT):
                                sp = pssp.tile([P, 2 * FB], FP32, tag="sp")
                                nc.tensor.matmul(
                                    sp[:, 0:FB], qkT[0:HD, KT + pg, ds(mc * P, P)],
                                    qkT[0:HD, pg, sl], start=True, stop=True)
                                nc.tensor.matmul(
                                    sp[:, FB:2 * FB], qkT[HD:P, KT + pg, ds(mc * P, P)],
                                    qkT[HD:P, pg, sl], start=True, stop=True)
                                e = exp_p.tile([P, 2 * FB], BF16, tag="e")
                                nc.scalar.activation(
                                    out=e, in_=sp, func=AF.Exp, scale=1.0, bias=zero_t)
                                es[mc] = e
                                if mc >= 1:
                                    emit_avs(mc - 1)
                            emit_avs(NT - 1)

                            # stage AV out of PSUM so the banks free immediately
                            avs0 = avsp.tile([HD + 1, FB], FP32, tag="avs")
                            avs1 = avsp.tile([HD + 1, FB], FP32, tag="avs")
                            nc.vector.tensor_copy(out=avs0, in_=av0[0:HD + 1, :])
                            nc.vector.tensor_copy(out=avs1, in_=av1[0:HD + 1, :])
                            staged.append((avs0, avs1))
                        return staged

                    def emit_att_norm(pg, staged):
                        # normalize the staged AV tiles; emitted AFTER rope(pg)
                        # so the DMA-latency-bound chain here doesn't block
                        # rope's DVE ops in the in-order queues
                        for nb in range(NB):
                            sl = ds(nb * FB, FB)
                            avs0, avs1 = staged[nb]
                            # spread the two [1,512] den rows across 64 DVE
                            # lanes (SBUF->SBUF reshape DMA) so one reciprocal
                            # covers both heads at 16 elems/lane
                            dg = rcp.tile([HD, 16], FP32, tag="dg")
                            nc.sync.dma_start(out=dg[0:32, :], in_=avs0[HD:HD + 1, :])
                            nc.sync.dma_start(out=dg[32:HD, :], in_=avs1[HD:HD + 1, :])
                            rg = rcp.tile([HD, 16], FP32, tag="rg")
                            nc.vector.reciprocal(out=rg, in_=dg)
                            nc.sync.dma_start(
                                out=rec_d[2 * pg:2 * pg + 1, sl], in_=rg[0:32, :])
                            nc.sync.dma_start(
                                out=rec_d[2 * pg + 1:2 * pg + 2, sl], in_=rg[32:HD, :])
                            R0 = rp.tile([HD, FB], FP32, tag="R")
                            R1 = rp.tile([HD, FB], FP32, tag="R")
                            nc.sync.dma_start(
                                out=R0,
                                in_=rec_d[2 * pg:2 * pg + 1, sl].broadcast_to([HD, FB]))
                            nc.sync.dma_start(
                                out=R1,
                                in_=rec_d[2 * pg + 1:2 * pg + 2, sl].broadcast_to([HD, FB]))
                            nc.vector.tensor_mul(
                                out=oT[0:HD, pg, sl], in0=avs0[0:HD, :], in1=R0)
                            ot1 = otp.tile([HD, FB], BF16, tag="ot1")
                            nc.vector.tensor_mul(
                                out=ot1, in0=avs1[0:HD, :], in1=R1)
                            nc.sync.dma_start(out=oT[HD:P, pg, sl], in_=ot1)

                    wos_all = []

                    for pg in range(KT):
                        sg_tiles = emit_proj_stats(pg)
                        # att(pg-1) matmuls before rope(pg): they are ready
                        # (inputs roped last iteration) while rope(pg) waits
                        # on the stats->Ln->Exp->DVE chain; the normalize
                        # tail goes after rope so its DMA round-trips don't
                        # block rope's DVE ops
                        staged = emit_att_mm(pg - 1) if pg >= 1 else None
                        rs_tiles = emit_stats_exp(sg_tiles)
                        emit_rope(pg, rs_tiles)
                        if staged is not None:
                            emit_att_norm(pg - 1, staged)
                    # prefetch output-projection weights under the last att
                    for ob in range(NB):
                        for k in range(KT):
                            wo = wop.tile([P, FB], BF16, tag="wo")
                            nc.sync.dma_start(
                                out=wo, in_=wout_d[ds(k * P, P), ds(ob * FB, FB)])
                            wos_all.append(wo)
                    staged = emit_att_mm(KT - 1)
                    emit_att_norm(KT - 1, staged)

            # -------- Phase 4: output projection (oT resident in SBUF) ----
            # ob interleaved inside nch so the 4MB of output DMA spreads
            # evenly instead of bunching at the end
            with tc.tile_pool(name="oout", bufs=3) as ooutp, \
                 tc.tile_pool(name="pso", bufs=3, space="PSUM") as pso:
                for nch in range(NT):
                    for ob in range(NB):
                        po = pso.tile([P, FB], FP32, tag="po")
                        for k in range(KT):
                            nc.tensor.matmul(
                                po, oT[:, k, ds(nch * P, P)], wos_all[ob * KT + k],
                                start=(k == 0),
                                stop=(zero_bias and k == KT - 1))
                        if not zero_bias:
                            nc.tensor.matmul(po, ones1, bout_t[:, ds(ob * FB, FB)],
                                             start=False, stop=True)
                        osb = ooutp.tile([P, FB], FP32, tag="osb")
                        if ob % 2 == 0:
                            nc.scalar.copy(out=osb, in_=po)
                        else:
                            nc.vector.tensor_copy(out=osb, in_=po)
                        nc.scalar.dma_start(
                            out=out[ds(nch * P, P), ds(ob * FB, FB)], in_=osb)

    nc.compile()
    return nc


def _host_inputs(Wqkv, bqkv, Wout, bout, q_scale, k_scale):
    import ml_dtypes
    BF = ml_dtypes.bfloat16

    cosF, sinF = _build_tables()

    ident = np.eye(P, dtype=np.float32)
    ones1 = np.ones((1, P), np.float32)

    # Fold q/k_scale into the Q/K projection columns; the RMSNorm variance of
    # the *unscaled* q is then recovered with a 1/scale^2-weighted reduction.
    qs = q_scale.astype(np.float32)
    ks = k_scale.astype(np.float32)
    W = Wqkv.astype(np.float32).copy()
    b = bqkv.astype(np.float32).copy()
    qcol = np.tile(qs, H)      # [D] scale per q feature
    kcol = np.tile(ks, H)
    W[:, 0:D] *= qcol[None, :]
    W[:, D:2 * D] *= kcol[None, :]
    b[0:D] *= qcol
    b[D:2 * D] *= kcol

    # wqk_cols[t, ki, ko, f] = W[ko*128+ki, t*128+perm(f)] -- q/k feature
    # rows in interleaved rotate-half pair order
    perm = np.array([(f // HD) * HD + _perm_d(f) for f in range(P)])
    wqk = np.ascontiguousarray(
        W[:, :2 * D].reshape(KT, P, 2 * KT, P).transpose(2, 1, 0, 3))[:, :, :, perm]
    wqk = np.ascontiguousarray(wqk)

    # block-diagonal stationary for broadcast q sumsq: col j weighted by
    # 1/(HD*scale^2) over the rows of j's head half
    iq = 1.0 / (qs * qs)
    iq_rows = np.array([iq[_perm_d(r)] for r in range(P)], np.float32)
    ones2qw = np.zeros((P, P), np.float32)
    for j in range(P):
        if j < HD:
            ones2qw[0:HD, j] = iq_rows[0:HD] / HD
        else:
            ones2qw[HD:P, j] = iq_rows[HD:P] / HD

    # same but sumsq-weighted (no /HD): 1/sqrt(sumsq) = 1/(8 sigma_k) folds
    # the hd^-0.5 softmax scale into kT
    ik = 1.0 / (ks * ks)
    ik_rows = np.array([ik[_perm_d(r)] for r in range(P)], np.float32)
    ones2kw = np.zeros((P, P), np.float32)
    for j in range(P):
        if j < HD:
            ones2kw[0:HD, j] = ik_rows[0:HD]
        else:
            ones2kw[HD:P, j] = ik_rows[HD:P]

    bqkv_cols = np.ascontiguousarray(
        b[:2 * D].reshape(2 * KT, P).T[perm, :]).astype(np.float32)

    return {
        "wqk_cols": wqk.astype(BF),
        "wv": np.ascontiguousarray(W[:, 2 * D:]).astype(BF),
        "wout": Wout.astype(np.float32).astype(BF),
        "bqkv_cols": bqkv_cols,
        "bqkv_v": b[2 * D:].reshape(1, D).astype(BF),
        "bout_r": bout.reshape(1, D).astype(np.float32).astype(BF),
        "cosf": cosF.astype(BF), "sinf": sinF.astype(BF),
        "ident": ident, "identb": ident.astype(BF),
        "ones2qw": ones2qw.astype(BF), "ones2kw": ones2kw.astype(BF),
        "ones1": ones1.astype(BF),
    }


def _get_built(zero_bias):
    global _BUILT
    if not isinstance(_BUILT, dict):
        _BUILT = {}
    if zero_bias not in _BUILT:
        _BUILT[zero_bias] = _build_program(zero_bias=zero_bias)
    return _BUILT[zero_bias]


def kernel(x, Wqkv, bqkv, Wout, bout, q_scale, k_scale, _trace=False):
    from concourse.bass_utils import run_bass_kernel_spmd

    x = np.asarray(x, dtype=np.float32)
    shared = _host_inputs(np.asarray(Wqkv, np.float32), np.asarray(bqkv, np.float32),
                          np.asarray(Wout, np.float32), np.asarray(bout, np.float32),
                          np.asarray(q_scale, np.float32), np.asarray(k_scale, np.float32))
    import ml_dtypes
    xb = np.ascontiguousarray(x).astype(ml_dtypes.bfloat16)
    in_maps = [dict(shared, x=xb[c]) for c in range(B)]
    zb = bool(not np.any(np.asarray(bqkv)) and not np.any(np.asarray(bout)))
    nc = _get_built(zb)
    res = run_bass_kernel_spmd(nc, in_maps, core_ids=list(range(B)), trace=_trace)
    out = np.stack([res.results[c]["out"] for c in range(B)], axis=0)
    kernel.last_exec_time_ns = res.exec_time_ns
    kernel.last_results = res
    return out
